# revision 18
# baseline (speedup 1.0000x reference)
"""MoE kernel, column-split expert pairing across core pairs (Trainium2).

Experts are paired (heaviest with lightest).  Each pair (A, B) maps to two
cores; BOTH cores receive ALL of the pair's tokens, but each core computes
only HALF of the DFF dimension (16 of 32 f-chunks) for both experts:

    core(p, half):  h = gelu(x @ W1[:, half] + b1[half])   [DFF/2, capP]
                    y_partial = h @ W2[half, :]             [D, capP]

The host adds the two partial y's (the DFF contraction is the only
cross-column coupling; gelu is elementwise per f-column, so the split is
exact).  Per-core token count is (cA+cB) ~ 1024+pad, per-core f-range is
halved, so the PE work per core is (cA+cB)/2 full-MLP equivalents -- load
balance within a pair is PERFECT regardless of dispatch skew, and weight
DMA per core is identical to plain expert-parallel (each core reads half
of two experts' weights = one expert's worth).

Device structure per core (uniform SPMD program):
  segment A (cap s_hi, the pair's heavier expert) then segment B (s_lo),
  each processed in blocks of <=2 chunks of <=512 tokens through
  phase 1 (x @ W1-half, gelu) and phase 2 (h @ W2-half).

Other optimizations (carried from the tuned single-expert kernel):
  per-f-chunk 256KB w1 slab streaming, w2 JIT streaming on both HWDGE
  rings, PE warm-up matmuls during the DMA fill, fp16 partial outputs,
  slim kernel teardown.

Self-contained: hardcodes all shapes from the problem spec.
"""

import os
import sys
from contextlib import ExitStack

import numpy as np

for _p in ("/opt/trn_rl_repo",):
    if _p not in sys.path:
        sys.path.insert(0, _p)

import concourse.bass as bass  # noqa: E402
import concourse.tile as tile  # noqa: E402
from concourse import mybir  # noqa: E402
from concourse.bass_utils import run_bass_kernel_spmd  # noqa: E402


def _patched_drain_and_barrier(self, tick_clock, wait_clock):
    from concourse.vector_clock import ScopedClock

    nc = self.nc
    probe = nc.sync.nop(nofuse=True)
    wait_clock.add_sem_waits(probe.ins, ScopedClock({None: tick_clock.global_clock}))
    si = probe.ins.sync_info
    waits = list(si.on_wait) if si and si.on_wait else []
    probe.ins.sync_info = mybir.SyncInfo(on_wait=waits[:1], on_update=[])
    for w in waits[1:]:
        n = nc.sync.nop(nofuse=True)
        n.ins.sync_info = mybir.SyncInfo(on_wait=[w], on_update=[])

    nc.sync.drain()
    nc.all_engine_barrier()
    assert self.sems is not None
    popped = nc._tile_sem_poison_stack.pop()
    assert popped is self._sem_poison


tile.TileContext._drain_and_barrier = _patched_drain_and_barrier


def _split_excess_sync_waits(nc, max_waits=1):
    """This walrus build only encodes one sem wait per instruction.  Hoist
    excess waits onto same-engine nops inserted immediately before."""
    for f in nc.m.functions:
        for bb in f.blocks:
            out = []
            for inst in bb.instructions:
                si = inst.sync_info
                if si and si.on_wait and len(si.on_wait) > max_waits:
                    waits = list(si.on_wait)
                    for i in range(max_waits, len(waits), max_waits):
                        n = mybir.InstNoOp(
                            name=f"{inst.name}-waitsplit-{i}", ins=[], outs=[]
                        )
                        n.engine = inst.engine
                        n.sync_info = mybir.SyncInfo(
                            on_wait=waits[i : i + max_waits], on_update=[]
                        )
                        out.append(n)
                    inst.sync_info = mybir.SyncInfo(
                        on_wait=waits[:max_waits], on_update=list(si.on_update or [])
                    )
                out.append(inst)
            bb.instructions[:] = out


NUM_EXPERTS = 8
D = 1024
DFF = 4096
N_CORES = 8
KD = D // 128  # 8 k-chunks for matmul 1
FC = DFF // 128  # 32 f-chunks total
FCH = FC // 2  # 16 f-chunks per core (its DFF half)
DM = D // 128  # 8 output chunks

F32 = mybir.dt.float32
F16 = mybir.dt.float16

LAST_EXEC_NS = None
LAST_RESULT = None

_NC_CACHE = {}


def _seg_plan(max_count):
    """Chunking for one segment: n chunks of equal mult-8 size <= 512."""
    n_chunks = max(1, -(-max_count // 512))
    chunk = max(8, -(-(-(-max_count // n_chunks)) // 8) * 8)
    return chunk, n_chunks


def _dm_schedule(dm_per_pass):
    sched = [dm_per_pass] * (DM // dm_per_pass)
    sched[-1:] = {1: [1], 2: [1, 1], 4: [2, 1, 1]}[sched[-1]]
    assert sum(sched) == DM
    return sched


def _seg_sched(n_chunks):
    dm_per_pass = max(1, 4 // min(n_chunks, 2))
    return _dm_schedule(dm_per_pass)


def _n_slabs(sched):
    return sum(FCH // (8 // dm_n) for dm_n in sched)


def _build_nc(a_chunk, na, b_chunk, nb):
    segs = [(a_chunk, na), (b_chunk, nb)]
    caps = [a_chunk * na, b_chunk * nb]
    cap = sum(caps)
    scheds = [_seg_sched(na), _seg_sched(nb)]
    slab_base = [0, _n_slabs(scheds[0])]
    n_w2_slabs = slab_base[1] + _n_slabs(scheds[1])
    x_cols = KD * cap

    nc = bass.Bass()
    xT = nc.declare_dram_parameter("xT", [128, x_cols], F16, isOutput=False)
    w1 = nc.declare_dram_parameter("w1", [128, 2 * FCH * KD * 128], F16,
                                   isOutput=False)
    w2 = nc.declare_dram_parameter("w2", [n_w2_slabs * 128, KD * 128], F16,
                                   isOutput=False)
    b1 = nc.declare_dram_parameter("b1", [128, 2 * FCH], F32, isOutput=False)
    b2 = nc.declare_dram_parameter("b2", [128, 2 * DM], F32, isOutput=False)
    yT = nc.declare_dram_parameter("yT", [128, DM * cap], F16, isOutput=True)

    gelu = mybir.ActivationFunctionType.Gelu_apprx_tanh
    w2p = w2.rearrange("(q p) d -> q p d", p=128)

    # Build the block list: per segment, blocks of <=2 chunks.
    # Each block: (seg, chunk_size, n_block_chunks, tok0, x_col0)
    blocks = []
    tok0 = 0
    for seg, (chunk, n_chunks) in enumerate(segs):
        c0 = 0
        while c0 < n_chunks:
            ncb = min(2, n_chunks - c0)
            blocks.append((seg, chunk, ncb, tok0 + c0 * chunk))
            c0 += ncb
        tok0 += caps[seg]

    with ExitStack() as ctx:
        tc = ctx.enter_context(tile.TileContext(nc))
        bpool = ctx.enter_context(tc.tile_pool(name="bias", bufs=1))
        xpool = ctx.enter_context(tc.tile_pool(name="xT", bufs=1))
        hpool = ctx.enter_context(tc.tile_pool(name="h", bufs=FCH))
        w1pool = ctx.enter_context(tc.tile_pool(name="w1", bufs=6))
        w2pool = ctx.enter_context(tc.tile_pool(name="w2", bufs=1))
        ypool = ctx.enter_context(tc.tile_pool(name="y", bufs=4))

        # NOTE: no PE warm-up matmuls.  Measured on HW: a warm PE running
        # full-rate during the full-bandwidth startup DMA burst trips the
        # chip's power limiter into a global /1.2 downclock for the rest
        # of the kernel.  The natural cold-clock ramp of the first real
        # matmuls keeps PE power low exactly while the DMA burst runs.

        # x tiles: one per (block, chunk-in-block), loaded per chunk.  The
        # first chunk is split across both HWDGE rings for a fast start.
        # Only block 0's x loads are issued up front: later blocks' x rides
        # the scalar ring 30+us before it's needed, keeping the startup
        # window clear for the w1 slab stream (measured: a 2.3us PE stall
        # around f=4 when all x chunks loaded at the start).
        xts = {}  # (block_index, c) -> tile
        xcol0 = {}
        xcol = 0
        for bi, (seg, chunk, ncb, _tok0) in enumerate(blocks):
            xcol0[bi] = xcol
            xcol += ncb * KD * chunk

        def _load_x(bi):
            seg, chunk, ncb, _tok0 = blocks[bi]
            xcol = xcol0[bi]
            for c in range(ncb):
                xt = xpool.tile([128, KD * chunk], F16, name=f"x{bi}_{c}",
                                tag=f"x{bi}_{c}")
                if bi == 0 and c == 0:
                    half = KD * chunk // 2
                    nc.sync.dma_start(xt[:, :half], xT[:, xcol : xcol + half])
                    nc.scalar.dma_start(
                        xt[:, half:], xT[:, xcol + half : xcol + KD * chunk]
                    )
                else:
                    nc.scalar.dma_start(
                        xt[:], xT[:, xcol : xcol + KD * chunk]
                    )
                xts[(bi, c)] = xt
                xcol += KD * chunk

        _load_x(0)

        b1_sb = bpool.tile([128, 2 * FCH], F32, tag="b1")
        nc.scalar.dma_start(b1_sb[:], b1[:, :])
        b2_sb = bpool.tile([128, 2 * DM], F32, tag="b2")
        nc.scalar.dma_start(b2_sb[:], b2[:, :])

        # w1 slab loading: slab (seg, f) is a [128, 1024] DMA.
        slab_of = {}

        def _load_w1_slab(seg, f):
            w = w1pool.tile([128, KD * 128], F16, name="w1s", tag="w1s")
            off = (seg * FCH + f) * KD * 128
            nc.sync.dma_start(w[:], w1[:, off : off + KD * 128])
            slab_of[(seg, f)] = w

        # w2 JIT streaming with 4-slab prefetch, alternating HWDGE rings.
        w2ts = [None] * n_w2_slabs

        def _load_w2(q):
            if q >= n_w2_slabs or w2ts[q] is not None:
                return
            w2t = w2pool.tile([128, KD * 128], F16, name="w2t",
                              tag=f"w2_{q % 8}")
            (nc.sync if q % 2 == 0 else nc.scalar).dma_start(w2t[:], w2p[q])
            w2ts[q] = w2t

        # Cross-block w1 slab prefetch order: flatten (block, f).
        slab_seq = []
        for bi, (seg, chunk, ncb, _t) in enumerate(blocks):
            # Segments with several blocks re-stream w1 per block; each
            # block contributes its full f-range to the stream sequence.
            for f in range(FCH):
                slab_seq.append((seg, f))
        seq_pos = 0
        for _ in range(min(5, len(slab_seq))):
            _load_w1_slab(*slab_seq[seq_pos])
            seq_pos += 1

        for bi, (seg, chunk, ncb, btok0) in enumerate(blocks):
            sched = scheds[seg]
            # ---- phase 1: h = gelu(x @ W1half + b1), feature-major ----
            hs = []
            with tc.tile_pool(name=f"p1_{bi}", bufs=4, space="PSUM") as p1pool:
                for f in range(FCH):
                    if seq_pos < len(slab_seq):
                        _load_w1_slab(*slab_seq[seq_pos])
                        seq_pos += 1
                    slab = slab_of.pop((seg, f))
                    h = hpool.tile([128, ncb * chunk], F16, name="h",
                                   tag=f"h{seg}")
                    for c in range(ncb):
                        ps = p1pool.tile([128, chunk], F32, name="ps", tag="ps")
                        for k in range(KD):
                            nc.tensor.matmul(
                                ps[:, :],
                                slab[:, k * 128 : (k + 1) * 128],
                                xts[(bi, c)][:, k * chunk : (k + 1) * chunk],
                                start=(k == 0),
                                stop=(k == KD - 1),
                            )
                        nc.scalar.activation(
                            h[:, c * chunk : (c + 1) * chunk],
                            ps[:, :],
                            gelu,
                            bias=b1_sb[:, seg * FCH + f : seg * FCH + f + 1],
                            scale=1.0,
                        )
                    hs.append(h)

            # ---- phase 2: y_partial = W2half @ h + b2 ----
            if bi + 1 < len(blocks):
                _load_x(bi + 1)
            for q in range(4):
                _load_w2(slab_base[seg] + q)
            # Phase-2 PSUM is split into two 4-bank pools: even passes on
            # the RIGHT side of PSUM (banks disjoint from the left-side p1
            # pool, so pass 0 starts without waiting for phase 1's last
            # activations to free banks), odd passes on the left, allocated
            # lazily at pass 1 when phase 1 has long drained.  Pool-overlap
            # dependencies are pool-granular, so a single 8-bank pool would
            # re-introduce the p1 dependency for pass 0.
            p2a = tc.alloc_tile_pool(name=f"p2a_{bi}", bufs=1, space="PSUM",
                                     side="right")
            p2b = None
            try:
                qrow = slab_base[seg]
                dm_lo = 0
                for pz, dm_n in enumerate(sched):
                    if pz == 1:
                        p2b = tc.alloc_tile_pool(name=f"p2b_{bi}", bufs=1,
                                                 space="PSUM")
                    pool = p2a if pz % 2 == 0 else p2b
                    fbn = 8 // dm_n
                    yps = {}
                    for dl in range(dm_n):
                        for c in range(ncb):
                            yps[(dl, c)] = pool.tile(
                                [128, chunk],
                                F32,
                                name=f"yp{pz % 2}_{dl}_{c}",
                                tag=f"yp{pz % 2}_{dl}_{c}",
                            )
                    for fq in range(FCH // fbn):
                        _load_w2(qrow + 4)
                        w2t = w2ts[qrow]
                        qrow += 1
                        for fb in range(fbn):
                            f = fq * fbn + fb
                            for dl in range(dm_n):
                                for c in range(ncb):
                                    nc.tensor.matmul(
                                        yps[(dl, c)][:, :],
                                        w2t[
                                            :,
                                            (fb * dm_n + dl) * 128 : (fb * dm_n + dl + 1) * 128,
                                        ],
                                        hs[f][:, c * chunk : (c + 1) * chunk],
                                        start=(f == 0),
                                        stop=(f == FCH - 1),
                                    )
                    for dl in range(dm_n):
                        dm = dm_lo + dl
                        yt = ypool.tile([128, 2 * chunk], F16, name="yt",
                                        tag="yt")
                        for c in range(ncb):
                            nc.vector.tensor_scalar_add(
                                yt[:, c * chunk : (c + 1) * chunk],
                                yps[(dl, c)][:, :],
                                b2_sb[:, seg * DM + dm : seg * DM + dm + 1],
                            )
                            nc.sync.dma_start(
                                yT[
                                    :,
                                    dm * cap + btok0 + c * chunk : dm * cap + btok0 + (c + 1) * chunk,
                                ],
                                yt[:, c * chunk : (c + 1) * chunk],
                            )
                    dm_lo += dm_n
            finally:
                if p2b is not None:
                    p2b.release()
                p2a.release()
                # Blocks beyond the first within a segment would need w2
                # re-streamed; reset the cache for them.
                if bi + 1 < len(blocks) and blocks[bi + 1][0] == seg:
                    for q in range(slab_base[seg], qrow):
                        w2ts[q] = None

    _split_excess_sync_waits(nc)
    _strip_const_memsets(nc)
    return nc


def _strip_const_memsets(nc):
    """The bass preamble memsets four unused constant tiles on gpsimd; they
    are this program's first 'useful' instructions and extend the profiled
    exec window ~1.2us before the first real DMA trigger.  No instruction
    reads them (verified); drop them."""
    for f in nc.m.functions:
        for bb in f.blocks:
            bb.instructions[:] = [
                i
                for i in bb.instructions
                if not (
                    isinstance(i, mybir.InstMemset)
                    and i.outs
                    and "const-" in str(getattr(i.outs[0], "memref", ""))
                )
            ]


def _pack_x_half(x_a, x_b, a_chunk, na, b_chunk, nb):
    """Chunk-major pack of the pair's tokens: A tokens then B tokens, each
    segment padded to its capacity and split into k-major chunk blocks."""
    out = []
    for x_seg, chunk, n_chunks in ((x_a, a_chunk, na), (x_b, b_chunk, nb)):
        capS = chunk * n_chunks
        xT = np.zeros((D, capS), dtype=np.float16)
        if x_seg.shape[0]:
            xT[:, : x_seg.shape[0]] = x_seg.T
        v = xT.reshape(KD, 128, n_chunks, chunk).transpose(1, 2, 0, 3)
        out.append(v.reshape(128, n_chunks * KD * chunk))
    return np.ascontiguousarray(np.concatenate(out, axis=1))


def _pack_w1_half(w1h):
    """[1024, 2048] f-half -> per-f-chunk slabs [128, FCH*KD*128]."""
    w = w1h.reshape(KD, 128, FCH, 128)
    packed = w.transpose(1, 2, 0, 3)  # [p, f, k, j]
    return np.ascontiguousarray(packed.reshape(128, FCH * KD * 128)).astype(
        np.float16
    )


def _pack_w2_half(w2h, sched):
    """[2048, 1024] row-half -> [128, 1024] slabs in consumption order."""
    w = w2h.reshape(FCH, 128, DM, 128)  # [f, p, dm, d2]
    slabs = []
    dm_lo = 0
    for dm_n in sched:
        fbn = 8 // dm_n
        for fq in range(FCH // fbn):
            slab = np.empty((128, 1024), np.float32)
            for fb in range(fbn):
                f = fq * fbn + fb
                for dl in range(dm_n):
                    lo = (fb * dm_n + dl) * 128
                    slab[:, lo : lo + 128] = w[f, :, dm_lo + dl, :]
            slabs.append(slab)
        dm_lo += dm_n
    return np.concatenate(slabs, axis=0).astype(np.float16)


def _enable_trace_hooks():
    import types

    if "antenv.axon_hooks" not in sys.modules:
        mod = types.ModuleType("antenv.axon_hooks")
        mod._hook = None

        def set_axon_ntff_profile_hook(h):
            mod._hook = h

        def get_axon_ntff_profile_hook():
            return mod._hook

        mod.set_axon_ntff_profile_hook = set_axon_ntff_profile_hook
        mod.get_axon_ntff_profile_hook = get_axon_ntff_profile_hook
        sys.modules["antenv.axon_hooks"] = mod
        import antenv

        antenv.axon_hooks = mod
    import antenv.axon_hooks as ah

    if ah.get_axon_ntff_profile_hook() is None:
        from trn_agent_boot.trn_boot import _ntff_profile_via_ctypes

        ah.set_axon_ntff_profile_hook(
            _ntff_profile_via_ctypes("/opt/axon/libaxon_pjrt.so")
        )
    import concourse.bass_utils as bu

    bu.upload_artifacts = lambda tmpdir: "local://skipped"


def kernel(inputs, w1, b1, w2, b2, dispatch_order):
    global LAST_EXEC_NS, LAST_RESULT

    inputs = np.asarray(inputs, dtype=np.float32)
    w1 = np.asarray(w1, dtype=np.float32)
    b1 = np.asarray(b1, dtype=np.float32)
    w2 = np.asarray(w2, dtype=np.float32)
    b2 = np.asarray(b2, dtype=np.float32)
    disp = np.asarray(dispatch_order).astype(np.int64)

    B, S, _ = inputs.shape
    T = B * S
    x = inputs.reshape(T, D)

    order = np.argsort(disp, kind="stable")
    counts = np.bincount(disp, minlength=NUM_EXPERTS)
    starts = np.zeros(NUM_EXPERTS + 1, dtype=np.int64)
    np.cumsum(counts, out=starts[1:])

    # Pair heaviest with lightest.
    by_count = np.argsort(-counts, kind="stable")
    pairs = [(int(by_count[i]), int(by_count[7 - i])) for i in range(4)]
    s_hi = max(int(counts[a]) for a, _ in pairs)
    s_lo = max(int(counts[b]) for _, b in pairs)
    a_chunk, na = _seg_plan(max(8, s_hi))
    b_chunk, nb = _seg_plan(max(8, s_lo))

    key = (a_chunk, na, b_chunk, nb)
    if key not in _NC_CACHE:
        _NC_CACHE[key] = _build_nc(*key)
    nc = _NC_CACHE[key]

    capA = a_chunk * na
    capB = b_chunk * nb
    cap = capA + capB
    schedA = _seg_sched(na)
    schedB = _seg_sched(nb)

    toks_of = [order[starts[e] : starts[e + 1]] for e in range(NUM_EXPERTS)]

    in_maps = []
    for p, (ea, eb) in enumerate(pairs):
        xpack = _pack_x_half(
            x[toks_of[ea]], x[toks_of[eb]], a_chunk, na, b_chunk, nb
        )
        for half in range(2):
            lo, hi = half * (DFF // 2), (half + 1) * (DFF // 2)
            w1p = np.concatenate(
                [_pack_w1_half(w1[ea][:, lo:hi]),
                 _pack_w1_half(w1[eb][:, lo:hi])], axis=1
            )
            w2pk = np.concatenate(
                [_pack_w2_half(w2[ea][lo:hi, :], schedA),
                 _pack_w2_half(w2[eb][lo:hi, :], schedB)], axis=0
            )
            b1p = np.concatenate(
                [np.ascontiguousarray(b1[ea][lo:hi].reshape(FCH, 128).T),
                 np.ascontiguousarray(b1[eb][lo:hi].reshape(FCH, 128).T)],
                axis=1,
            )
            if half == 0:
                b2p = np.concatenate(
                    [np.ascontiguousarray(b2[ea].reshape(DM, 128).T),
                     np.ascontiguousarray(b2[eb].reshape(DM, 128).T)], axis=1
                )
            else:
                b2p = np.zeros((128, 2 * DM), np.float32)
            in_maps.append(
                {"xT": xpack, "w1": w1p, "w2": w2pk, "b1": b1p, "b2": b2p}
            )

    trace = os.environ.get("MOE_TRACE") == "1"
    kwargs = {}
    if trace:
        _enable_trace_hooks()
        kwargs["trace"] = True
        tmpdir = os.environ.get("MOE_TRACE_DIR")
        if tmpdir:
            os.makedirs(tmpdir, exist_ok=True)
            kwargs["tmpdir"] = tmpdir

    res = run_bass_kernel_spmd(nc, in_maps, list(range(N_CORES)), **kwargs)
    LAST_RESULT = res
    LAST_EXEC_NS = res.exec_time_ns

    out = np.empty((T, D), dtype=np.float32)
    for p, (ea, eb) in enumerate(pairs):
        y0 = res.results[2 * p]["yT"].astype(np.float32)
        y1 = res.results[2 * p + 1]["yT"].astype(np.float32)
        ye = (y0 + y1).reshape(128, DM, cap)
        ta, tb = toks_of[ea], toks_of[eb]
        if len(ta):
            out[ta] = ye[:, :, : len(ta)].transpose(2, 1, 0).reshape(len(ta), D)
        if len(tb):
            out[tb] = (
                ye[:, :, capA : capA + len(tb)].transpose(2, 1, 0)
                .reshape(len(tb), D)
            )
    return out.reshape(B, S, D)


# revision 19
# speedup vs baseline: 1.2021x; 1.2021x over previous
"""MoE kernel, column-split expert pairing across core pairs (Trainium2).

Experts are paired (heaviest with lightest).  Each pair (A, B) maps to two
cores; BOTH cores receive ALL of the pair's tokens, but each core computes
only HALF of the DFF dimension (16 of 32 f-chunks) for both experts:

    core(p, half):  h = gelu(x @ W1[:, half] + b1[half])   [DFF/2, capP]
                    y_partial = h @ W2[half, :]             [D, capP]

The host adds the two partial y's (the DFF contraction is the only
cross-column coupling; gelu is elementwise per f-column, so the split is
exact).  Per-core token count is (cA+cB) ~ 1024+pad, per-core f-range is
halved, so the PE work per core is (cA+cB)/2 full-MLP equivalents -- load
balance within a pair is PERFECT regardless of dispatch skew, and weight
DMA per core is identical to plain expert-parallel (each core reads half
of two experts' weights = one expert's worth).

Device structure per core (uniform SPMD program):
  segment A (cap s_hi, the pair's heavier expert) then segment B (s_lo),
  each processed in blocks of <=2 chunks of <=512 tokens through
  phase 1 (x @ W1-half, gelu) and phase 2 (h @ W2-half).

Other optimizations (carried from the tuned single-expert kernel):
  per-f-chunk 256KB w1 slab streaming, w2 JIT streaming on both HWDGE
  rings, deferred per-block x loads, split-side phase-2 PSUM pools (no
  phase-transition stalls), fp16 partial outputs, slim kernel teardown,
  and NO pre-warmup (a warm PE during the startup DMA burst can trip the
  chip power limiter into a global /1.2 downclock).

Self-contained: hardcodes all shapes from the problem spec.
"""

import os
import sys
from contextlib import ExitStack

import numpy as np

for _p in ("/opt/trn_rl_repo",):
    if _p not in sys.path:
        sys.path.insert(0, _p)

import concourse.bass as bass  # noqa: E402
import concourse.tile as tile  # noqa: E402
from concourse import mybir  # noqa: E402
from concourse.bass_utils import run_bass_kernel_spmd  # noqa: E402


def _patched_drain_and_barrier(self, tick_clock, wait_clock):
    from concourse.vector_clock import ScopedClock

    nc = self.nc
    probe = nc.sync.nop(nofuse=True)
    wait_clock.add_sem_waits(probe.ins, ScopedClock({None: tick_clock.global_clock}))
    si = probe.ins.sync_info
    waits = list(si.on_wait) if si and si.on_wait else []
    probe.ins.sync_info = mybir.SyncInfo(on_wait=waits[:1], on_update=[])
    for w in waits[1:]:
        n = nc.sync.nop(nofuse=True)
        n.ins.sync_info = mybir.SyncInfo(on_wait=[w], on_update=[])

    nc.sync.drain()
    nc.all_engine_barrier()
    assert self.sems is not None
    popped = nc._tile_sem_poison_stack.pop()
    assert popped is self._sem_poison


tile.TileContext._drain_and_barrier = _patched_drain_and_barrier


def _split_excess_sync_waits(nc, max_waits=1):
    """This walrus build only encodes one sem wait per instruction.  Hoist
    excess waits onto same-engine nops inserted immediately before."""
    for f in nc.m.functions:
        for bb in f.blocks:
            out = []
            for inst in bb.instructions:
                si = inst.sync_info
                if si and si.on_wait and len(si.on_wait) > max_waits:
                    waits = list(si.on_wait)
                    for i in range(max_waits, len(waits), max_waits):
                        n = mybir.InstNoOp(
                            name=f"{inst.name}-waitsplit-{i}", ins=[], outs=[]
                        )
                        n.engine = inst.engine
                        n.sync_info = mybir.SyncInfo(
                            on_wait=waits[i : i + max_waits], on_update=[]
                        )
                        out.append(n)
                    inst.sync_info = mybir.SyncInfo(
                        on_wait=waits[:max_waits], on_update=list(si.on_update or [])
                    )
                out.append(inst)
            bb.instructions[:] = out


NUM_EXPERTS = 8
D = 1024
DFF = 4096
N_CORES = 8
KD = D // 128  # 8 k-chunks for matmul 1
FC = DFF // 128  # 32 f-chunks total
FCH = FC // 2  # 16 f-chunks per core (its DFF half)
DM = D // 128  # 8 output chunks

F32 = mybir.dt.float32
F16 = mybir.dt.float16

LAST_EXEC_NS = None
LAST_RESULT = None

_NC_CACHE = {}


def _seg_plan(max_count):
    """Chunking for one segment: n chunks of equal mult-8 size <= 512."""
    n_chunks = max(1, -(-max_count // 512))
    chunk = max(8, -(-(-(-max_count // n_chunks)) // 8) * 8)
    return chunk, n_chunks


def _dm_schedule(dm_per_pass):
    sched = [dm_per_pass] * (DM // dm_per_pass)
    sched[-1:] = {1: [1], 2: [1, 1], 4: [2, 1, 1]}[sched[-1]]
    assert sum(sched) == DM
    return sched


def _seg_sched(n_chunks):
    dm_per_pass = max(1, 4 // min(n_chunks, 2))
    return _dm_schedule(dm_per_pass)


def _n_slabs(sched):
    return sum(FCH // (8 // dm_n) for dm_n in sched)


def _build_nc(a_chunk, na, b_chunk, nb):
    segs = [(a_chunk, na), (b_chunk, nb)]
    caps = [a_chunk * na, b_chunk * nb]
    cap = sum(caps)
    scheds = [_seg_sched(na), _seg_sched(nb)]
    slab_base = [0, _n_slabs(scheds[0])]
    n_w2_slabs = slab_base[1] + _n_slabs(scheds[1])
    x_cols = KD * cap

    nc = bass.Bass()
    xT = nc.declare_dram_parameter("xT", [128, x_cols], F16, isOutput=False)
    w1 = nc.declare_dram_parameter("w1", [128, 2 * FCH * KD * 128], F16,
                                   isOutput=False)
    w2 = nc.declare_dram_parameter("w2", [n_w2_slabs * 128, KD * 128], F16,
                                   isOutput=False)
    b1 = nc.declare_dram_parameter("b1", [128, 2 * FCH], F32, isOutput=False)
    b2 = nc.declare_dram_parameter("b2", [128, 2 * DM], F32, isOutput=False)
    yT = nc.declare_dram_parameter("yT", [128, DM * cap], F16, isOutput=True)

    gelu = mybir.ActivationFunctionType.Gelu_apprx_tanh
    w2p = w2.rearrange("(q p) d -> q p d", p=128)

    # Build the block list: per segment, blocks of <=2 chunks.
    # Each block: (seg, chunk_size, n_block_chunks, tok0, x_col0)
    blocks = []
    tok0 = 0
    for seg, (chunk, n_chunks) in enumerate(segs):
        c0 = 0
        while c0 < n_chunks:
            ncb = min(2, n_chunks - c0)
            blocks.append((seg, chunk, ncb, tok0 + c0 * chunk))
            c0 += ncb
        tok0 += caps[seg]

    with ExitStack() as ctx:
        tc = ctx.enter_context(tile.TileContext(nc))
        bpool = ctx.enter_context(tc.tile_pool(name="bias", bufs=1))
        xpool = ctx.enter_context(tc.tile_pool(name="xT", bufs=1))
        hpool = ctx.enter_context(tc.tile_pool(name="h", bufs=FCH))
        w1pool = ctx.enter_context(tc.tile_pool(name="w1", bufs=6))
        w2pool = ctx.enter_context(tc.tile_pool(name="w2", bufs=1))
        ypool = ctx.enter_context(tc.tile_pool(name="y", bufs=4))

        # NOTE: no PE warm-up matmuls.  Measured on HW: a warm PE running
        # full-rate during the full-bandwidth startup DMA burst trips the
        # chip's power limiter into a global /1.2 downclock for the rest
        # of the kernel.  The natural cold-clock ramp of the first real
        # matmuls keeps PE power low exactly while the DMA burst runs.

        # x tiles: one per (block, chunk-in-block), loaded per chunk.  The
        # first chunk is split across both HWDGE rings for a fast start.
        # Only block 0's x loads are issued up front: later blocks' x rides
        # the scalar ring 30+us before it's needed, keeping the startup
        # window clear for the w1 slab stream (measured: a 2.3us PE stall
        # around f=4 when all x chunks loaded at the start).
        xts = {}  # (block_index, c) -> tile
        xcol0 = {}
        xcol = 0
        for bi, (seg, chunk, ncb, _tok0) in enumerate(blocks):
            xcol0[bi] = xcol
            xcol += ncb * KD * chunk

        def _load_x(bi):
            seg, chunk, ncb, _tok0 = blocks[bi]
            xcol = xcol0[bi]
            for c in range(ncb):
                xt = xpool.tile([128, KD * chunk], F16, name=f"x{bi}_{c}",
                                tag=f"x{bi}_{c}")
                if bi == 0 and c == 0:
                    half = KD * chunk // 2
                    nc.sync.dma_start(xt[:, :half], xT[:, xcol : xcol + half])
                    nc.scalar.dma_start(
                        xt[:, half:], xT[:, xcol + half : xcol + KD * chunk]
                    )
                else:
                    nc.scalar.dma_start(
                        xt[:], xT[:, xcol : xcol + KD * chunk]
                    )
                xts[(bi, c)] = xt
                xcol += KD * chunk

        _load_x(0)

        b1_sb = bpool.tile([128, 2 * FCH], F32, tag="b1")
        nc.scalar.dma_start(b1_sb[:], b1[:, :])
        b2_sb = bpool.tile([128, 2 * DM], F32, tag="b2")
        nc.scalar.dma_start(b2_sb[:], b2[:, :])

        # w1 slab loading: slab (seg, f) is a [128, 1024] DMA.
        slab_of = {}

        def _load_w1_slab(seg, f):
            w = w1pool.tile([128, KD * 128], F16, name="w1s", tag="w1s")
            off = (seg * FCH + f) * KD * 128
            nc.sync.dma_start(w[:], w1[:, off : off + KD * 128])
            slab_of[(seg, f)] = w

        # w2 JIT streaming with 4-slab prefetch, alternating HWDGE rings.
        w2ts = [None] * n_w2_slabs

        def _load_w2(q):
            if q >= n_w2_slabs or w2ts[q] is not None:
                return
            w2t = w2pool.tile([128, KD * 128], F16, name="w2t",
                              tag=f"w2_{q % 8}")
            (nc.sync if q % 2 == 0 else nc.scalar).dma_start(w2t[:], w2p[q])
            w2ts[q] = w2t

        # Cross-block w1 slab prefetch order: flatten (block, f).
        slab_seq = []
        for bi, (seg, chunk, ncb, _t) in enumerate(blocks):
            # Segments with several blocks re-stream w1 per block; each
            # block contributes its full f-range to the stream sequence.
            for f in range(FCH):
                slab_seq.append((seg, f))
        seq_pos = 0
        for _ in range(min(5, len(slab_seq))):
            _load_w1_slab(*slab_seq[seq_pos])
            seq_pos += 1

        for bi, (seg, chunk, ncb, btok0) in enumerate(blocks):
            sched = scheds[seg]
            # ---- phase 1: h = gelu(x @ W1half + b1), feature-major ----
            hs = []
            with tc.tile_pool(name=f"p1_{bi}", bufs=4, space="PSUM") as p1pool:
                for f in range(FCH):
                    if seq_pos < len(slab_seq):
                        _load_w1_slab(*slab_seq[seq_pos])
                        seq_pos += 1
                    slab = slab_of.pop((seg, f))
                    h = hpool.tile([128, ncb * chunk], F16, name="h",
                                   tag=f"h{seg}")
                    for c in range(ncb):
                        ps = p1pool.tile([128, chunk], F32, name="ps", tag="ps")
                        for k in range(KD):
                            nc.tensor.matmul(
                                ps[:, :],
                                slab[:, k * 128 : (k + 1) * 128],
                                xts[(bi, c)][:, k * chunk : (k + 1) * chunk],
                                start=(k == 0),
                                stop=(k == KD - 1),
                            )
                        nc.scalar.activation(
                            h[:, c * chunk : (c + 1) * chunk],
                            ps[:, :],
                            gelu,
                            bias=b1_sb[:, seg * FCH + f : seg * FCH + f + 1],
                            scale=1.0,
                        )
                    hs.append(h)

            # ---- phase 2: y_partial = W2half @ h + b2 ----
            if bi + 1 < len(blocks):
                _load_x(bi + 1)
            for q in range(4):
                _load_w2(slab_base[seg] + q)
            # Phase-2 PSUM is split into two 4-bank pools: even passes on
            # the RIGHT side of PSUM (banks disjoint from the left-side p1
            # pool, so pass 0 starts without waiting for phase 1's last
            # activations to free banks), odd passes on the left, allocated
            # lazily at pass 1 when phase 1 has long drained.  Pool-overlap
            # dependencies are pool-granular, so a single 8-bank pool would
            # re-introduce the p1 dependency for pass 0.
            p2a = tc.alloc_tile_pool(name=f"p2a_{bi}", bufs=1, space="PSUM",
                                     side="right")
            p2b = None
            try:
                qrow = slab_base[seg]
                dm_lo = 0
                for pz, dm_n in enumerate(sched):
                    if pz == 1:
                        p2b = tc.alloc_tile_pool(name=f"p2b_{bi}", bufs=1,
                                                 space="PSUM")
                    pool = p2a if pz % 2 == 0 else p2b
                    fbn = 8 // dm_n
                    yps = {}
                    for dl in range(dm_n):
                        for c in range(ncb):
                            yps[(dl, c)] = pool.tile(
                                [128, chunk],
                                F32,
                                name=f"yp{pz % 2}_{dl}_{c}",
                                tag=f"yp{pz % 2}_{dl}_{c}",
                            )
                    for fq in range(FCH // fbn):
                        _load_w2(qrow + 4)
                        w2t = w2ts[qrow]
                        qrow += 1
                        for fb in range(fbn):
                            f = fq * fbn + fb
                            for dl in range(dm_n):
                                for c in range(ncb):
                                    nc.tensor.matmul(
                                        yps[(dl, c)][:, :],
                                        w2t[
                                            :,
                                            (fb * dm_n + dl) * 128 : (fb * dm_n + dl + 1) * 128,
                                        ],
                                        hs[f][:, c * chunk : (c + 1) * chunk],
                                        start=(f == 0),
                                        stop=(f == FCH - 1),
                                    )
                    for dl in range(dm_n):
                        dm = dm_lo + dl
                        yt = ypool.tile([128, 2 * chunk], F16, name="yt",
                                        tag="yt")
                        for c in range(ncb):
                            nc.vector.tensor_scalar_add(
                                yt[:, c * chunk : (c + 1) * chunk],
                                yps[(dl, c)][:, :],
                                b2_sb[:, seg * DM + dm : seg * DM + dm + 1],
                            )
                            nc.sync.dma_start(
                                yT[
                                    :,
                                    dm * cap + btok0 + c * chunk : dm * cap + btok0 + (c + 1) * chunk,
                                ],
                                yt[:, c * chunk : (c + 1) * chunk],
                            )
                    dm_lo += dm_n
            finally:
                if p2b is not None:
                    p2b.release()
                p2a.release()
                # Blocks beyond the first within a segment would need w2
                # re-streamed; reset the cache for them.
                if bi + 1 < len(blocks) and blocks[bi + 1][0] == seg:
                    for q in range(slab_base[seg], qrow):
                        w2ts[q] = None

    _split_excess_sync_waits(nc)
    _strip_const_memsets(nc)
    return nc


def _strip_const_memsets(nc):
    """The bass preamble memsets four unused constant tiles on gpsimd; they
    are this program's first 'useful' instructions and extend the profiled
    exec window ~1.2us before the first real DMA trigger.  No instruction
    reads them (verified); drop them."""
    for f in nc.m.functions:
        for bb in f.blocks:
            bb.instructions[:] = [
                i
                for i in bb.instructions
                if not (
                    isinstance(i, mybir.InstMemset)
                    and i.outs
                    and "const-" in str(getattr(i.outs[0], "memref", ""))
                )
            ]


def _pack_x_half(x_a, x_b, a_chunk, na, b_chunk, nb):
    """Chunk-major pack of the pair's tokens: A tokens then B tokens, each
    segment padded to its capacity and split into k-major chunk blocks."""
    out = []
    for x_seg, chunk, n_chunks in ((x_a, a_chunk, na), (x_b, b_chunk, nb)):
        capS = chunk * n_chunks
        xT = np.zeros((D, capS), dtype=np.float16)
        if x_seg.shape[0]:
            xT[:, : x_seg.shape[0]] = x_seg.T
        v = xT.reshape(KD, 128, n_chunks, chunk).transpose(1, 2, 0, 3)
        out.append(v.reshape(128, n_chunks * KD * chunk))
    return np.ascontiguousarray(np.concatenate(out, axis=1))


def _pack_w1_half(w1h):
    """[1024, 2048] f-half -> per-f-chunk slabs [128, FCH*KD*128]."""
    w = w1h.reshape(KD, 128, FCH, 128)
    packed = w.transpose(1, 2, 0, 3)  # [p, f, k, j]
    return np.ascontiguousarray(packed.reshape(128, FCH * KD * 128)).astype(
        np.float16
    )


def _pack_w2_half(w2h, sched):
    """[2048, 1024] row-half -> [128, 1024] slabs in consumption order."""
    w = w2h.reshape(FCH, 128, DM, 128)  # [f, p, dm, d2]
    slabs = []
    dm_lo = 0
    for dm_n in sched:
        fbn = 8 // dm_n
        for fq in range(FCH // fbn):
            slab = np.empty((128, 1024), np.float32)
            for fb in range(fbn):
                f = fq * fbn + fb
                for dl in range(dm_n):
                    lo = (fb * dm_n + dl) * 128
                    slab[:, lo : lo + 128] = w[f, :, dm_lo + dl, :]
            slabs.append(slab)
        dm_lo += dm_n
    return np.concatenate(slabs, axis=0).astype(np.float16)


def _enable_trace_hooks():
    import types

    if "antenv.axon_hooks" not in sys.modules:
        mod = types.ModuleType("antenv.axon_hooks")
        mod._hook = None

        def set_axon_ntff_profile_hook(h):
            mod._hook = h

        def get_axon_ntff_profile_hook():
            return mod._hook

        mod.set_axon_ntff_profile_hook = set_axon_ntff_profile_hook
        mod.get_axon_ntff_profile_hook = get_axon_ntff_profile_hook
        sys.modules["antenv.axon_hooks"] = mod
        import antenv

        antenv.axon_hooks = mod
    import antenv.axon_hooks as ah

    if ah.get_axon_ntff_profile_hook() is None:
        from trn_agent_boot.trn_boot import _ntff_profile_via_ctypes

        ah.set_axon_ntff_profile_hook(
            _ntff_profile_via_ctypes("/opt/axon/libaxon_pjrt.so")
        )
    import concourse.bass_utils as bu

    bu.upload_artifacts = lambda tmpdir: "local://skipped"


def kernel(inputs, w1, b1, w2, b2, dispatch_order):
    global LAST_EXEC_NS, LAST_RESULT

    inputs = np.asarray(inputs, dtype=np.float32)
    w1 = np.asarray(w1, dtype=np.float32)
    b1 = np.asarray(b1, dtype=np.float32)
    w2 = np.asarray(w2, dtype=np.float32)
    b2 = np.asarray(b2, dtype=np.float32)
    disp = np.asarray(dispatch_order).astype(np.int64)

    B, S, _ = inputs.shape
    T = B * S
    x = inputs.reshape(T, D)

    order = np.argsort(disp, kind="stable")
    counts = np.bincount(disp, minlength=NUM_EXPERTS)
    starts = np.zeros(NUM_EXPERTS + 1, dtype=np.int64)
    np.cumsum(counts, out=starts[1:])

    # Pair heaviest with lightest.
    by_count = np.argsort(-counts, kind="stable")
    pairs = [(int(by_count[i]), int(by_count[7 - i])) for i in range(4)]
    s_hi = max(int(counts[a]) for a, _ in pairs)
    s_lo = max(int(counts[b]) for _, b in pairs)
    a_chunk, na = _seg_plan(max(8, s_hi))
    b_chunk, nb = _seg_plan(max(8, s_lo))

    key = (a_chunk, na, b_chunk, nb)
    if key not in _NC_CACHE:
        _NC_CACHE[key] = _build_nc(*key)
    nc = _NC_CACHE[key]

    capA = a_chunk * na
    capB = b_chunk * nb
    cap = capA + capB
    schedA = _seg_sched(na)
    schedB = _seg_sched(nb)

    toks_of = [order[starts[e] : starts[e + 1]] for e in range(NUM_EXPERTS)]

    in_maps = []
    for p, (ea, eb) in enumerate(pairs):
        xpack = _pack_x_half(
            x[toks_of[ea]], x[toks_of[eb]], a_chunk, na, b_chunk, nb
        )
        for half in range(2):
            lo, hi = half * (DFF // 2), (half + 1) * (DFF // 2)
            w1p = np.concatenate(
                [_pack_w1_half(w1[ea][:, lo:hi]),
                 _pack_w1_half(w1[eb][:, lo:hi])], axis=1
            )
            w2pk = np.concatenate(
                [_pack_w2_half(w2[ea][lo:hi, :], schedA),
                 _pack_w2_half(w2[eb][lo:hi, :], schedB)], axis=0
            )
            b1p = np.concatenate(
                [np.ascontiguousarray(b1[ea][lo:hi].reshape(FCH, 128).T),
                 np.ascontiguousarray(b1[eb][lo:hi].reshape(FCH, 128).T)],
                axis=1,
            )
            if half == 0:
                b2p = np.concatenate(
                    [np.ascontiguousarray(b2[ea].reshape(DM, 128).T),
                     np.ascontiguousarray(b2[eb].reshape(DM, 128).T)], axis=1
                )
            else:
                b2p = np.zeros((128, 2 * DM), np.float32)
            in_maps.append(
                {"xT": xpack, "w1": w1p, "w2": w2pk, "b1": b1p, "b2": b2p}
            )

    trace = os.environ.get("MOE_TRACE") == "1"
    kwargs = {}
    if trace:
        _enable_trace_hooks()
        kwargs["trace"] = True
        tmpdir = os.environ.get("MOE_TRACE_DIR")
        if tmpdir:
            os.makedirs(tmpdir, exist_ok=True)
            kwargs["tmpdir"] = tmpdir

    res = run_bass_kernel_spmd(nc, in_maps, list(range(N_CORES)), **kwargs)
    LAST_RESULT = res
    LAST_EXEC_NS = res.exec_time_ns

    out = np.empty((T, D), dtype=np.float32)
    for p, (ea, eb) in enumerate(pairs):
        y0 = res.results[2 * p]["yT"].astype(np.float32)
        y1 = res.results[2 * p + 1]["yT"].astype(np.float32)
        ye = (y0 + y1).reshape(128, DM, cap)
        ta, tb = toks_of[ea], toks_of[eb]
        if len(ta):
            out[ta] = ye[:, :, : len(ta)].transpose(2, 1, 0).reshape(len(ta), D)
        if len(tb):
            out[tb] = (
                ye[:, :, capA : capA + len(tb)].transpose(2, 1, 0)
                .reshape(len(tb), D)
            )
    return out.reshape(B, S, D)
